# revision 1
# baseline (speedup 1.0000x reference)
"""Trainium2 Bass kernel for nn_MultiHeadRelativeAttention (S=256, E=1024, H=16).

Sharding: tensor-parallel over the head dimension. Each of the 8 cores owns 2
heads (a contiguous 128-wide slice of the E-sized head space), computes its
slice of the projections, scores, softmax and attention output, and produces a
full-shape (256, 1024) partial of the output projection. The host sums the 8
partials (the output projection contracts over the head dim, so the sum is the
all-reduce).

Device decomposition per core:
  - projections computed transposed: nqT/keyT/relT (128 dh, 256 s) via PE with
    host-pre-transposed weights + activations; value (256 j, 128 f) directly.
  - a2 = nq_i . s_k[i] (the 256MB stream, the flop/byte~1 hot loop): per pair
    of query rows {m, m+64}, one accumulating N=512 bf16 matmul (one PSUM
    bank). The stationary operand is a 128-col window into a zero-spaced
    buffer B holding nq column (m + 64f) at position 128m + 64f; window
    B[:, 127m : 127m+128] then has exactly 2 live columns at relative
    positions m+64f, so output row band [64f, 64f+64) is valid only in column
    copy f of the (128, 512) PSUM score tile; all other contributions are
    zero or land in never-read copies. a1 opens the groups with 2 strip
    matmuls at (row 64f, col 256f).
  - a3 = rel-shifted nq.rel: block matmul, then Transformer-XL shift via a
    DRAM bounce: store rows at stride 512 into a zero-padded scratch, read
    back with an affine AP (row stride 511, offset 255-128*ib) which lands
    tril(shift(r)) exactly (pad region supplies the zeros).
  - mask folded as multiply by (1-mask)*scaling: masked logits -> 0 and
    exp(0) == exp(1e-30) == 1.0 in fp32, identical to the reference.
  - softmax without max-subtraction (logits are O(10), safe in fp32).
"""

import sys

if "/opt/trn_rl_repo" not in sys.path:
    sys.path.insert(0, "/opt/trn_rl_repo")

import numpy as np

import concourse.bass as bass
import concourse.mybir as mybir
import concourse.tile as tile
from concourse import bacc
from concourse.masks import make_identity

S = 256
E = 1024
H = 16
HD = 64
NCORES = 8
DHB = 128          # head-dim block per core (2 heads x 64)
SCALING = float(HD) ** -0.5

F32 = mybir.dt.float32
BF16 = mybir.dt.bfloat16
NPBF = np.dtype("bfloat16")

# Precision knob: keep the logits path (projections of q/k/p, a1, a3) in f32
# (PE 4 cyc/row, ~2.7e-3 rel err) vs bf16 everywhere (~6.2e-3 rel err).
LOGITS_F32 = True
LG_DT = F32 if LOGITS_F32 else BF16
LG_NP = np.float32 if LOGITS_F32 else NPBF


def emit(tc: tile.TileContext, t: dict):
    nc = tc.nc
    from contextlib import ExitStack

    ctx = ExitStack()
    const = ctx.enter_context(tc.tile_pool(name="const", bufs=1))
    skp = ctx.enter_context(tc.tile_pool(name="skp", bufs=3))
    work = ctx.enter_context(tc.tile_pool(name="work", bufs=3))
    psS = ctx.enter_context(tc.tile_pool(name="psS", bufs=2, space="PSUM"))
    psA = ctx.enter_context(tc.tile_pool(name="psA", bufs=1, space="PSUM"))
    psM = ctx.enter_context(tc.tile_pool(name="psM", bufs=2, space="PSUM"))

    skt_q = {}

    def load_skt(idx):
        ib, ch = divmod(idx, 4)
        skt = skp.tile([128, 2, 16, 256], BF16, tag="skt", name=f"skt{idx}")
        for f in range(2):
            i0 = 128 * ib + 16 * ch + 64 * f
            nc.sync.dma_start(out=skt[:, f, :, :],
                              in_=t["skT"][:, 256 * i0:256 * (i0 + 16)])
        skt_q[idx] = skt

    # prefetch the first s_k tiles; the s_k stream owns the Sync HWDGE ring
    for idx in range(3):
        load_skt(idx)

    # ---- constants (Scalar HWDGE ring, critical-path order) ----
    xq = const.tile([128, 8, 256], LG_DT, tag="xq")
    xk = const.tile([128, 8, 256], LG_DT, tag="xk")
    xp = const.tile([128, 8, 256], LG_DT, tag="xp")
    xv = const.tile([128, 8, 256], BF16, tag="xv")
    wq = const.tile([128, 8, 128], LG_DT, tag="wq")
    wk = const.tile([128, 8, 128], LG_DT, tag="wk")
    wr = const.tile([128, 8, 128], LG_DT, tag="wr")
    wv = const.tile([128, 8, 128], BF16, tag="wv")
    wo = const.tile([128, 1024], BF16, tag="wo")
    sq = const.tile([128, 1], F32, tag="sq")
    nc.scalar.dma_start(out=sq, in_=t["sq"])
    for sb, name in ((xq, "qT"), (wq, "WqT"), (xk, "kT"), (wk, "WkT"),
                     (xp, "pT"), (wr, "WrT"), (xv, "vT"), (wv, "WvT")):
        if name.startswith("W"):
            nc.scalar.dma_start(
                out=sb, in_=t[name].rearrange("(c p) d -> p c d", p=128))
        else:
            nc.scalar.dma_start(
                out=sb, in_=t[name].rearrange("(c p) s -> p c s", p=128))
    nc.scalar.dma_start(out=wo, in_=t["WoT"])

    ident = const.tile([128, 128], F32)
    make_identity(nc, ident)

    # zero-spaced stationary buffer, split across engines
    B = const.tile([128, 8192], BF16, tag="B")
    nc.vector.memset(B[:, 0:4096], 0.0)
    nc.gpsimd.memset(B[:, 4096:8192], 0.0)

    # zero the pad halves of the 4 a3 bounce scratches
    zt = const.tile([128, 256], F32, tag="zt")
    nc.vector.memset(zt, 0.0)
    for h in range(2):
        for ib in range(2):
            nc.scalar.dma_start(out=t[f"a3scr{h}{ib}"][:, 256:512], in_=zt)

    mnot = []
    for h in range(2):
        for ib in range(2):
            m = const.tile([128, 256], BF16, tag=f"mnot{h}{ib}")
            nc.scalar.dma_start(out=m, in_=t["mnot"][h, ib])
            mnot.append(m)

    # ---- projections (transposed): (128 dh, 256 s) ----
    def proj_T(wsb, xsb):
        ps = psM.tile([128, 256], F32, tag="pm", name="ps_proj")
        for c in range(8):
            nc.tensor.matmul(ps, wsb[:, c, :], xsb[:, c, :],
                             start=(c == 0), stop=(c == 7))
        return ps

    hs = [slice(64 * h, 64 * h + 64) for h in range(2)]

    # query projection first (only needs xq+wq), then scatter both B buffers
    nqT = const.tile([128, 256], LG_DT, tag="nqT")
    nc.vector.tensor_scalar_add(out=nqT, in0=proj_T(wq, xq), scalar1=sq)

    def scatter_B(ib):
        Bv = B.rearrange("p (m q) -> p m q", q=128)
        for f in range(2):
            nc.vector.tensor_copy(
                out=Bv[:, :, 64 * f],
                in_=nqT[:, 128 * ib + 64 * f:128 * ib + 64 * f + 64],
            )

    # score-tile state per i-block
    S_ps = [None, None]
    a1_ps = [None, None]
    a3s = [[None, None], [None, None]]

    def a2_chunk(ib, ch):
        """Stream one s_k chunk: 16 window-matmuls per head, N=512 each."""
        idx = 4 * ib + ch
        skt = skt_q.pop(idx)
        if idx + 3 < 8:
            load_skt(idx + 3)
        for h in range(2):
            for gg in range(16):
                m = 16 * ch + gg
                nc.tensor.matmul(
                    S_ps[ib][h][:, :],
                    B[hs[h], 127 * m:127 * m + 128],
                    skt[hs[h], :, gg, :],
                    start=(m == 0), stop=(m == 63),
                )

    def open_block(ib):
        S_ps[ib] = [psS.tile([128, 512], F32, tag=f"S{h}", name=f"S{ib}{h}")
                    for h in range(2)]

    def small_mms(ib, keyT, relT):
        """a1 + a3 for one i-block (tiny matmuls)."""
        a1_ps[ib] = []
        for h in range(2):
            pa = psA.tile([128, 256], F32, tag=f"a1{h}", name=f"a1_{ib}{h}")
            nc.tensor.matmul(pa, nqT[hs[h], 128 * ib:128 * ib + 128],
                             keyT[hs[h], :], start=True, stop=True)
            a1_ps[ib].append(pa)
        for h in range(2):
            ps = psM.tile([128, 256], F32, tag="pm", name="ps_a3")
            nc.tensor.matmul(ps, nqT[hs[h], 128 * ib:128 * ib + 128],
                             relT[hs[h], :], start=True, stop=True)
            raw = work.tile([128, 256], F32, tag="a3raw")
            nc.scalar.copy(out=raw, in_=ps)
            scr = t[f"a3scr{h}{ib}"]
            nc.scalar.dma_start(out=scr[:, 0:256], in_=raw)
            sh = work.tile([128, 256], F32, tag="a3s", name=f"a3s_{h}{ib}")
            nc.scalar.dma_start(
                out=sh,
                in_=bass.AP(tensor=scr.tensor, offset=scr.offset + 255 - 128 * ib,
                            ap=[[511, 128], [1, 256]]),
            )
            a3s[ib][h] = sh

    out_sb = [const.tile([128, 1024], F32, tag=f"out{ib}", name=f"out{ib}")
              for ib in range(2)]

    def block_tail(ib):
        """softmax + attn@v + output projection for a finished i-block."""
        attn = work.tile([128, 128], F32, tag="attn")
        for h in range(2):
            w1 = work.tile([128, 256], F32, tag="w1")
            for f in range(2):
                r = slice(64 * f, 64 * f + 64)
                nc.vector.tensor_add(out=w1[r, :],
                                     in0=S_ps[ib][h][r, 256 * f:256 * f + 256],
                                     in1=a3s[ib][h][r, :])
            w15 = work.tile([128, 256], F32, tag="w15")
            nc.vector.tensor_add(out=w15, in0=w1, in1=a1_ps[ib][h][:, :])
            w2 = work.tile([128, 256], F32, tag="w2")
            nc.vector.tensor_mul(out=w2, in0=w15, in1=mnot[2 * h + ib])
            ex = work.tile([128, 256], F32, tag="ex")
            nc.scalar.activation(out=ex, in_=w2,
                                 func=mybir.ActivationFunctionType.Exp, scale=1.0)
            den = work.tile([128, 1], F32, tag="den")
            nc.vector.reduce_sum(out=den, in_=ex, axis=mybir.AxisListType.X)
            rden = work.tile([128, 1], F32, tag="rden")
            nc.vector.reciprocal(out=rden, in_=den)
            sc = work.tile([128, 256], F32, tag="sc")
            nc.vector.tensor_scalar_mul(out=sc, in0=ex, scalar1=rden)

            av = psM.tile([128, 64], F32, tag="pm", name="ps_av")
            for jh in range(2):
                tp = psM.tile([128, 128], F32, tag="pm", name="ps_tr")
                nc.tensor.transpose(tp, sc[:, 128 * jh:128 * jh + 128], ident)
                st = work.tile([128, 128], BF16, tag="st")
                nc.scalar.copy(out=st, in_=tp)
                nc.tensor.matmul(av, st, value[jh][:, hs[h]],
                                 start=(jh == 0), stop=(jh == 1))
            nc.scalar.copy(out=attn[:, hs[h]], in_=av)

        tp = psM.tile([128, 128], F32, tag="pm", name="ps_atr")
        nc.tensor.transpose(tp, attn, ident)
        aT = work.tile([128, 128], BF16, tag="aT")
        nc.scalar.copy(out=aT, in_=tp)
        for nh in range(2):
            op = psM.tile([128, 512], F32, tag="pm", name="ps_out")
            nc.tensor.matmul(op, aT, wo[:, 512 * nh:512 * (nh + 1)],
                             start=True, stop=True)
            nc.scalar.copy(out=out_sb[ib][:, 512 * nh:512 * (nh + 1)], in_=op)
        nc.scalar.dma_start(out=t["outp"][128 * ib:128 * (ib + 1), :],
                            in_=out_sb[ib])

    # ---- PE-order schedule ----
    # remaining projections + a1/a3
    keyT = const.tile([128, 256], LG_DT, tag="keyT")
    nc.scalar.copy(out=keyT, in_=proj_T(wk, xk))
    relT = const.tile([128, 256], LG_DT, tag="relT")
    nc.scalar.copy(out=relT, in_=proj_T(wr, xp))
    value = []
    for jh in range(2):
        ps = psM.tile([128, 128], F32, tag="pm", name="ps_val")
        for c in range(8):
            nc.tensor.matmul(ps, xv[:, c, 128 * jh:128 * jh + 128],
                             wv[:, c, :], start=(c == 0), stop=(c == 7))
        vsb = const.tile([128, 128], BF16, tag=f"value{jh}", name=f"value{jh}")
        nc.scalar.copy(out=vsb, in_=ps)
        value.append(vsb)
    for ib in range(2):
        scatter_B(ib)
        open_block(ib)
        small_mms(ib, keyT, relT)
        for ch in range(4):
            a2_chunk(ib, ch)
        block_tail(ib)

    ctx.close()


def build():
    nc = bacc.Bacc("TRN2", target_bir_lowering=False, debug=False)
    t = {}

    def inp(name, shape, dt=F32):
        t[name] = nc.dram_tensor(name, list(shape), dt, kind="ExternalInput").ap()

    inp("skT", (128, S * S), BF16)
    for n in ("qT", "kT", "pT"):
        inp(n, (E, S), LG_DT)
    inp("vT", (E, S), BF16)
    for n in ("WqT", "WkT", "WrT"):
        inp(n, (E, DHB), LG_DT)
    inp("WvT", (E, DHB), BF16)
    inp("WoT", (DHB, E), BF16)
    inp("sq", (DHB, 1))
    inp("mnot", (2, 2, 128, 256), BF16)
    for h in range(2):
        for ib in range(2):
            t[f"a3scr{h}{ib}"] = nc.dram_tensor(
                f"a3scr{h}{ib}", [128, 512], F32).ap()
    t["outp"] = nc.dram_tensor("outp", [S, E], F32, kind="ExternalOutput").ap()

    with tile.TileContext(nc) as tc:
        emit(tc, t)
    nc.compile()
    return nc


def make_in_maps(inputs: dict) -> list[dict]:
    q = np.asarray(inputs["q"], np.float32)
    k = np.asarray(inputs["k"], np.float32)
    v = np.asarray(inputs["v"], np.float32)
    p = np.asarray(inputs["p"], np.float32)
    mask = np.asarray(inputs["mask"])
    s_q = np.asarray(inputs["s_q"], np.float32)
    s_k = np.asarray(inputs["s_k"], np.float32)
    Wq = np.asarray(inputs["Wq"], np.float32)
    Wk = np.asarray(inputs["Wk"], np.float32)
    Wv = np.asarray(inputs["Wv"], np.float32)
    Wr = np.asarray(inputs["Wr"], np.float32)
    Wo = np.asarray(inputs["Wo"], np.float32)

    qT = np.ascontiguousarray(q.T).astype(LG_NP)
    kT = np.ascontiguousarray(k.T).astype(LG_NP)
    pT = np.ascontiguousarray(p.T).astype(LG_NP)
    vT = np.ascontiguousarray(v.T).astype(NPBF)

    maps = []
    for c in range(NCORES):
        rows = slice(c * DHB, (c + 1) * DHB)
        skT = np.ascontiguousarray(s_k[:, rows].T).astype(NPBF)
        mn = np.empty((2, 2, 128, 256), np.float32)
        for h in range(2):
            mh = mask[2 * c + h]
            for ib in range(2):
                mn[h, ib] = (1.0 - mh[128 * ib:128 * (ib + 1)].astype(np.float32)
                             ) * SCALING
        maps.append({
            "skT": skT,
            "qT": qT, "kT": kT, "pT": pT, "vT": vT,
            "WqT": np.ascontiguousarray(Wq[rows].T).astype(LG_NP),
            "WkT": np.ascontiguousarray(Wk[rows].T).astype(LG_NP),
            "WrT": np.ascontiguousarray(Wr[rows].T).astype(LG_NP),
            "WvT": np.ascontiguousarray(Wv[rows].T).astype(NPBF),
            "WoT": np.ascontiguousarray(Wo[:, rows].T).astype(NPBF),
            "sq": np.ascontiguousarray(s_q[0, rows][:, None]),
            "mnot": mn.astype(NPBF),
        })
    return maps


_NC = None


def kernel(**inputs) -> np.ndarray:
    global _NC
    from concourse.bass_utils import run_bass_kernel_spmd

    if _NC is None:
        _NC = build()
    maps = make_in_maps(inputs)
    res = run_bass_kernel_spmd(_NC, maps, list(range(NCORES)))
    out = np.zeros((S, E), np.float32)
    for r in res.results:
        out += np.asarray(r["outp"], np.float32)
    return out


if __name__ == "__main__":
    nc = build()
    print("build ok")



# revision 3
# speedup vs baseline: 1.3105x; 1.3105x over previous
"""Trainium2 Bass kernel for nn_MultiHeadRelativeAttention (S=256, E=1024, H=16).

Sharding: tensor-parallel over heads. Each of 8 cores owns 2 heads (a 128-wide
d-slice), computes its projections, scores, softmax, attention and a partial
output projection; the host sums the 8 partials (the Wo contraction over d is
the all-reduce).

v1 redesign vs baseline:
  - all inputs host-prelaid in exact SBUF tile layout (contiguous DMAs; the
    baseline's rearranged gathers cost ~5.4us DGE-gen each and serialized a
    28us dead start).
  - a2 uses both heads per matmul: 128-partition contraction, one N=256
    matmul per query row i. Stationary is a 128-col window into a zero-spaced
    buffer B (window stride 127) holding nq[h0,i] at col 128w (partitions
    0-63) and nq[h1,i] at col 128w+64 (partitions 64-127); output rows r<64
    are head-0 scores, r>=64 head-1. Half the PE time of the 64-partition
    variant.
  - a1 folded into the score PSUM: two K=64 matmuls (nq slice x keyT) open
    each group's accumulation with start=True over the two partition halves.
  - groups of 64 i-rows; 4 groups, PSUM bank each; tails (softmax, attn@v,
    out-proj) interleave into the next group's a2 stream on the PE queue.
  - a3 (rel-shift) via DRAM bounce in bf16, read back with the affine AP
    (row stride 511) that lands tril(shift(.)) exactly.
  - PE warmup matmuls at t=0 against the identity to climb the p-state ramp
    while the first DMAs land.
"""

import sys

if "/opt/trn_rl_repo" not in sys.path:
    sys.path.insert(0, "/opt/trn_rl_repo")

import numpy as np

import concourse.bass as bass
import concourse.mybir as mybir
import concourse.tile as tile
from concourse import bacc
from concourse.masks import make_identity

S = 256
E = 1024
H = 16
HD = 64
NCORES = 8
DHB = 128          # head-dim block per core (2 heads x 64)
SCALING = float(HD) ** -0.5

F32 = mybir.dt.float32
BF16 = mybir.dt.bfloat16
NPBF = np.dtype("bfloat16")

NCHUNK = 16        # s_k stream chunks (16 i-rows each)
CHI = 16           # i-rows per chunk


def emit(tc: tile.TileContext, t: dict):
    nc = tc.nc
    from contextlib import ExitStack

    ctx = ExitStack()
    const = ctx.enter_context(tc.tile_pool(name="const", bufs=1))
    skp = ctx.enter_context(tc.tile_pool(name="skp", bufs=6))
    work = ctx.enter_context(tc.tile_pool(name="work", bufs=3))
    psS = ctx.enter_context(tc.tile_pool(name="psS", bufs=3, space="PSUM"))
    psT = ctx.enter_context(tc.tile_pool(name="psT", bufs=2, space="PSUM"))
    psM = ctx.enter_context(tc.tile_pool(name="psM", bufs=2, space="PSUM"))

    # ---- s_k stream on the Sync ring ----
    skq = {}

    def load_chunk(k):
        st = skp.tile([128, CHI, 256], BF16, tag="skt", name=f"skt{k}")
        nc.sync.dma_start(out=st, in_=t["skT"][:, 4096 * k:4096 * (k + 1)])
        skq[k] = st

    for k in range(3):
        load_chunk(k)

    # ---- constants on the Scalar ring, critical-path order ----
    sq = const.tile([128, 1], F32, tag="sq")
    wq = const.tile([128, 8, 128], BF16, tag="wq")
    xq = const.tile([128, 8, 256], BF16, tag="xq")
    wk = const.tile([128, 8, 128], BF16, tag="wk")
    xk = const.tile([128, 8, 256], BF16, tag="xk")
    wr = const.tile([128, 8, 128], BF16, tag="wr")
    xp = const.tile([128, 8, 256], BF16, tag="xp")
    wv = const.tile([128, 8, 128], BF16, tag="wv")
    xv = const.tile([128, 8, 256], BF16, tag="xv")
    wo = const.tile([128, 1024], BF16, tag="wo")
    for sb, name in ((sq, "sq"), (wq, "wq"), (xq, "xq"), (wk, "wk"),
                     (xk, "xk"), (wr, "wr"), (xp, "xp"), (wv, "wv"),
                     (xv, "xv"), (wo, "wo")):
        nc.scalar.dma_start(out=sb, in_=t[name])
    mnot = []
    for g in range(4):
        m = const.tile([128, 256], BF16, tag=f"mnot{g}")
        nc.scalar.dma_start(out=m, in_=t["mnot"][g])
        mnot.append(m)

    # identity (bf16) for transposes + PE warmup
    ident = const.tile([128, 128], BF16, tag="ident")
    make_identity(nc, ident)
    warm = psM.tile([128, 128], BF16, tag="pm", name="warm")
    for _ in range(28):
        nc.tensor.matmul(warm, ident, ident, start=True, stop=True,
                         is_transpose=True, skip_group_check=True)

    # B buffers (zero-spaced stationary), zeroed once, live cols rewritten
    B = [const.tile([128, 8192], BF16, tag=f"B{i}", name=f"B{i}")
         for i in range(2)]
    Bv = [b.rearrange("p (w c) -> p w c", c=128) for b in B]
    nc.vector.memset(B[0], 0.0)
    nc.gpsimd.memset(B[1], 0.0)

    # zero pads of the a3 bounce scratches (one DMA from a zeroed tile)
    zt4 = const.tile([128, 4, 256], BF16, tag="zt4")
    nc.gpsimd.memset(zt4, 0.0)
    nc.gpsimd.dma_start(
        out=bass.AP(tensor=t["a3scr"].tensor, offset=t["a3scr"].offset + 256,
                    ap=[[512, 128], [65536, 4], [1, 256]]),
        in_=zt4)

    # ---- projections (transposed): (128 d, 256 s) ----
    def proj_T(wsb, xsb, name):
        ps = psM.tile([128, 512], F32, tag="pm", name=name)
        for c in range(8):
            nc.tensor.matmul(ps[:, 0:256], wsb[:, c, :], xsb[:, c, :],
                             start=(c == 0), stop=(c == 7))
        return ps

    nqT = const.tile([128, 256], BF16, tag="nqT")
    nc.vector.tensor_scalar_add(out=nqT, in0=proj_T(wq, xq, "ps_nq")[:, 0:256],
                                scalar1=sq)
    keyT = const.tile([128, 256], BF16, tag="keyT")
    nc.scalar.copy(out=keyT, in_=proj_T(wk, xk, "ps_key")[:, 0:256])

    def scatter_B(g):
        bb = Bv[g % 2]
        base = 64 * g
        nc.gpsimd.tensor_copy(out=bb[0:64, 0:64, 0],
                              in_=nqT[0:64, base:base + 64])
        nc.gpsimd.tensor_copy(out=bb[64:128, 0:64, 64],
                              in_=nqT[64:128, base:base + 64])

    S_ps = [None] * 4

    def open_group(g):
        base = 64 * g
        sp = psS.tile([128, 512], F32, tag="S", name=f"S{g}")
        S_ps[g] = sp
        for h in range(2):
            hsl = slice(64 * h, 64 * h + 64)
            nc.tensor.matmul(sp[hsl, 0:256], nqT[hsl, base:base + 64],
                             keyT[hsl, :], start=True, stop=False,
                             skip_group_check=True)

    def a2_run(g, w0, w1):
        """a2 matmuls for group g, window range [w0, w1)."""
        base = 64 * g
        bb = B[g % 2]
        sp = S_ps[g]
        for w in range(w0, w1):
            i = base + w
            k, r = divmod(i, CHI)
            st = skq[k]
            nc.tensor.matmul(sp[:, 0:256], bb[:, 127 * w:127 * w + 128],
                             st[:, r, :], start=False,
                             stop=(w == 63), skip_group_check=True)
            if r == CHI - 1:
                del skq[k]
                if k + 3 < NCHUNK:
                    load_chunk(k + 3)

    # ---- a3: small matmuls + DRAM bounce shift ----
    a3sb = [[None, None], [None, None]]

    def a3_emit(relT):
        for h in range(2):
            hsl = slice(64 * h, 64 * h + 64)
            for ib in range(2):
                ps = psM.tile([128, 512], F32, tag="pm", name=f"a3r{h}{ib}")
                nc.tensor.matmul(ps[:, 0:256], nqT[hsl, 128 * ib:128 * ib + 128],
                                 relT[hsl, :], start=True, stop=True)
                raw = work.tile([128, 256], BF16, tag="a3raw")
                nc.scalar.copy(out=raw, in_=ps[:, 0:256])
                scr = t["a3scr"]
                off0 = scr.offset + (2 * h + ib) * 65536
                nc.gpsimd.dma_start(
                    out=bass.AP(tensor=scr.tensor, offset=off0,
                                ap=[[512, 128], [1, 256]]),
                    in_=raw)
                sh = const.tile([128, 256], BF16, tag=f"a3s{h}{ib}")
                nc.gpsimd.dma_start(
                    out=sh,
                    in_=bass.AP(tensor=scr.tensor,
                                offset=off0 + 255 - 128 * ib,
                                ap=[[511, 128], [1, 256]]))
                a3sb[h][ib] = sh

    # ---- per-group tail ----
    value = [None, None]
    sc_t = [None] * 4
    stc_t = [None] * 4
    aT_t = [None] * 4
    op_t = [None] * 4

    def tail_vec(g):
        """softmax chain on DVE/Act for group g."""
        base = 64 * g
        sp = S_ps[g]
        w2 = work.tile([128, 256], F32, tag="w2")
        for h in range(2):
            hsl = slice(64 * h, 64 * h + 64)
            nc.vector.tensor_add(
                out=w2[hsl, :], in0=sp[hsl, 0:256],
                in1=a3sb[h][g // 2][64 * (g % 2):64 * (g % 2) + 64, :])
        w3 = work.tile([128, 256], F32, tag="w3")
        nc.vector.tensor_mul(out=w3, in0=w2, in1=mnot[g])
        ex = work.tile([128, 256], F32, tag="ex")
        nc.scalar.activation(out=ex, in_=w3,
                             func=mybir.ActivationFunctionType.Exp, scale=1.0)
        den = work.tile([128, 1], F32, tag="den")
        nc.vector.reduce_sum(out=den, in_=ex, axis=mybir.AxisListType.X)
        rden = work.tile([128, 1], F32, tag="rden")
        nc.vector.reciprocal(out=rden, in_=den)
        sc = work.tile([128, 256], BF16, tag="sc", name=f"sc{g}")
        nc.vector.tensor_scalar_mul(out=sc, in0=ex, scalar1=rden)
        sc_t[g] = sc

    def tail_tr(g):
        """transposes of sc (PE) + psum->sbuf copies."""
        sc = sc_t[g]
        stc = []
        for jb in range(2):
            tp = psT.tile([128, 128], BF16, tag="tp", name=f"tp{g}{jb}")
            nc.tensor.transpose(tp, sc[:, 128 * jb:128 * jb + 128], ident)
            st = work.tile([128, 128], BF16, tag="stc", name=f"stc{g}{jb}")
            nc.scalar.copy(out=st, in_=tp)
            stc.append(st)
        stc_t[g] = stc

    def tail_av(g):
        """attn@v transposed: avT (128 d, 64 i) psum + copy to bf16."""
        stc = stc_t[g]
        av = psM.tile([128, 512], F32, tag="pm", name=f"av{g}")
        for h in range(2):
            hsl = slice(64 * h, 64 * h + 64)
            for jb in range(2):
                nc.tensor.matmul(av[hsl, 0:64], value[jb][:, hsl],
                                 stc[jb][:, hsl], start=(jb == 0),
                                 stop=(jb == 1), skip_group_check=True)
        aT = work.tile([128, 64], BF16, tag="aT", name=f"aT{g}")
        nc.scalar.copy(out=aT, in_=av[:, 0:64])
        aT_t[g] = aT

    def tail_out(g):
        """output projection (64 i, 1024 e) + copy + DMA."""
        aT = aT_t[g]
        ops = []
        for eh in range(2):
            op = psM.tile([128, 512], F32, tag="pm", name=f"op{g}{eh}")
            nc.tensor.matmul(op[0:64, :], aT, wo[:, 512 * eh:512 * (eh + 1)],
                             start=True, stop=True, skip_group_check=True)
            ob = work.tile([64, 512], F32, tag="ob", name=f"ob{g}{eh}")
            nc.scalar.copy(out=ob, in_=op[0:64, :])
            nc.gpsimd.dma_start(
                out=t["outp"][64 * g:64 * g + 64, 512 * eh:512 * (eh + 1)],
                in_=ob)
            ops.append(op)
        op_t[g] = ops

    # ---- PE-order schedule ----
    scatter_B(0)
    open_group(0)
    a2_run(0, 0, 16)

    relT = const.tile([128, 256], BF16, tag="relT")
    nc.scalar.copy(out=relT, in_=proj_T(wr, xp, "ps_rel")[:, 0:256])
    a3_emit(relT)

    a2_run(0, 16, 32)

    # value projection: (128 j, 128 d) per j-block
    for jb in range(2):
        ps = psM.tile([128, 512], F32, tag="pm", name=f"ps_val{jb}")
        for c in range(8):
            nc.tensor.matmul(ps[:, 0:128], xv[:, c, 128 * jb:128 * jb + 128],
                             wv[:, c, :], start=(c == 0), stop=(c == 7))
        vsb = const.tile([128, 128], BF16, tag=f"value{jb}")
        nc.scalar.copy(out=vsb, in_=ps[:, 0:128])
        value[jb] = vsb

    a2_run(0, 32, 64)
    tail_vec(0)

    scatter_B(1)
    open_group(1)
    a2_run(1, 0, 16)
    tail_tr(0)
    a2_run(1, 16, 32)
    tail_av(0)
    a2_run(1, 32, 48)
    tail_out(0)
    a2_run(1, 48, 64)
    tail_vec(1)

    for g in (2, 3):
        scatter_B(g)
        open_group(g)
        a2_run(g, 0, 16)
        tail_tr(g - 1)
        a2_run(g, 16, 32)
        tail_av(g - 1)
        a2_run(g, 32, 48)
        tail_out(g - 1)
        a2_run(g, 48, 64)
        tail_vec(g)

    tail_tr(3)
    tail_av(3)
    tail_out(3)

    ctx.close()


def build():
    nc = bacc.Bacc("TRN2", target_bir_lowering=False, debug=False)
    t = {}

    def inp(name, shape, dt=BF16):
        t[name] = nc.dram_tensor(name, list(shape), dt, kind="ExternalInput").ap()

    inp("skT", (128, S * S))
    inp("sq", (128, 1), F32)
    for n in ("xq", "xk", "xp", "xv"):
        inp(n, (128, 8, 256))
    for n in ("wq", "wk", "wr", "wv"):
        inp(n, (128, 8, 128))
    inp("wo", (128, 1024))
    inp("mnot", (4, 128, 256))
    t["a3scr"] = nc.dram_tensor("a3scr", [4, 128, 512], BF16).ap()
    t["outp"] = nc.dram_tensor("outp", [S, E], F32, kind="ExternalOutput").ap()

    with tile.TileContext(nc) as tc:
        emit(tc, t)
    nc.compile()
    return nc


def make_in_maps(inputs: dict) -> list[dict]:
    q = np.asarray(inputs["q"], np.float32)
    k = np.asarray(inputs["k"], np.float32)
    v = np.asarray(inputs["v"], np.float32)
    p = np.asarray(inputs["p"], np.float32)
    mask = np.asarray(inputs["mask"])
    s_q = np.asarray(inputs["s_q"], np.float32)
    s_k = np.asarray(inputs["s_k"], np.float32)
    Wq = np.asarray(inputs["Wq"], np.float32)
    Wk = np.asarray(inputs["Wk"], np.float32)
    Wv = np.asarray(inputs["Wv"], np.float32)
    Wr = np.asarray(inputs["Wr"], np.float32)
    Wo = np.asarray(inputs["Wo"], np.float32)

    def actT(x):
        # (S, E) -> (128, 8, 256) bf16: out[p, c, s] = x[s, 128c+p]
        return np.ascontiguousarray(
            x.T.reshape(8, 128, 256).transpose(1, 0, 2)).astype(NPBF)

    xq, xk, xp, xv = actT(q), actT(k), actT(p), actT(v)

    maps = []
    for c in range(NCORES):
        rows = slice(c * DHB, (c + 1) * DHB)

        def wT(W):
            # (128, 8, 128): out[p, cc, m] = W[128c+m, 128cc+p]
            return np.ascontiguousarray(
                W[rows].T.reshape(8, 128, 128).transpose(1, 0, 2)).astype(NPBF)

        skT = np.ascontiguousarray(s_k[:, rows].T).astype(NPBF)
        mn = np.empty((4, 128, 256), np.float32)
        for g in range(4):
            for h in range(2):
                mn[g, 64 * h:64 * h + 64] = (
                    1.0 - mask[2 * c + h, 64 * g:64 * g + 64].astype(np.float32)
                ) * SCALING
        maps.append({
            "skT": skT,
            "xq": xq, "xk": xk, "xp": xp, "xv": xv,
            "wq": wT(Wq), "wk": wT(Wk), "wr": wT(Wr), "wv": wT(Wv),
            "wo": np.ascontiguousarray(Wo[:, rows].T).astype(NPBF),
            "sq": np.ascontiguousarray(s_q[0, rows][:, None]),
            "mnot": mn.astype(NPBF),
        })
    return maps


_NC = None


def kernel(**inputs) -> np.ndarray:
    global _NC
    from concourse.bass_utils import run_bass_kernel_spmd

    if _NC is None:
        _NC = build()
    maps = make_in_maps(inputs)
    res = run_bass_kernel_spmd(_NC, maps, list(range(NCORES)))
    out = np.zeros((S, E), np.float32)
    for r in res.results:
        out += np.asarray(r["outp"], np.float32)
    return out


if __name__ == "__main__":
    nc = build()
    print("build ok")


# revision 6
# speedup vs baseline: 1.3619x; 1.0392x over previous
"""Trainium2 Bass kernel for nn_MultiHeadRelativeAttention (S=256, E=1024, H=16).

Sharding: tensor-parallel over heads. Each of 8 cores owns 2 heads (a 128-wide
d-slice), computes its projections, scores, softmax, attention and a partial
output projection; the host sums the 8 partials (the Wo contraction over d is
the all-reduce).

Design (v2):
  - all inputs host-prelaid in exact SBUF tile layout (contiguous DMAs).
  - a2 with both heads per matmul: 128-partition contraction, one N=256
    matmul per query row. Stationary is a 128-col window (stride 127) into a
    zero-spaced buffer B holding nq[h0,i] at col 128w (partitions 0-63) and
    nq[h1,i] at col 128w+64 (partitions 64-127); PSUM rows 0-63 are head-0
    scores, 64-127 head-1. 4 groups of 64 rows, one PSUM bank each.
  - a1 folded into the score accumulation: two K=64 matmuls open each group.
  - a3 (Transformer-XL rel-shift) via a bf16 DRAM bounce (affine read-back,
    row stride 511, lands tril(shift(.))), then folded into the score PSUM
    with identity-stationary matmuls at the end of each group.
  - single B buffer: half-memsets at startup, live-column rewrites per group
    pipelined mid-stream (scatter of group g+1's first window half runs while
    group g streams its second half) -> no PE bubble at group boundaries.
  - DMA rings: sync = critical consts + s_k chunks + bounce + outputs;
    gpsimd = secondary consts; scalar(Act) ring carries no DMAs.
  - PE warmup transposes against the identity climb the p-state ramp during
    the initial DMA window.
"""

import sys

if "/opt/trn_rl_repo" not in sys.path:
    sys.path.insert(0, "/opt/trn_rl_repo")

import numpy as np

import concourse.bass as bass
import concourse.mybir as mybir
import concourse.tile as tile
from concourse import bacc
from concourse.masks import make_identity

S = 256
E = 1024
H = 16
HD = 64
NCORES = 8
DHB = 128
SCALING = float(HD) ** -0.5

F32 = mybir.dt.float32
BF16 = mybir.dt.bfloat16
NPBF = np.dtype("bfloat16")

NCHUNK = 16
CHI = 16


def emit(tc: tile.TileContext, t: dict):
    nc = tc.nc
    from contextlib import ExitStack

    ctx = ExitStack()
    const = ctx.enter_context(tc.tile_pool(name="const", bufs=1))
    skp = ctx.enter_context(tc.tile_pool(name="skp", bufs=6))
    work = ctx.enter_context(tc.tile_pool(name="work", bufs=3))
    psS = ctx.enter_context(tc.tile_pool(name="psS", bufs=3, space="PSUM"))
    psT = ctx.enter_context(tc.tile_pool(name="psT", bufs=2, space="PSUM"))
    psM = ctx.enter_context(tc.tile_pool(name="psM", bufs=2, space="PSUM"))

    # identity first: it gates the PE warmup
    ident = const.tile([128, 128], BF16, tag="ident")
    make_identity(nc, ident)

    # ---- critical consts + s_k stream on the Sync ring ----
    sq = const.tile([128, 1], F32, tag="sq")
    wq = const.tile([128, 8, 128], BF16, tag="wq")
    xq = const.tile([128, 8, 256], BF16, tag="xq")
    for sb, name in ((sq, "sq"), (wq, "wq"), (xq, "xq")):
        nc.sync.dma_start(out=sb, in_=t[name])

    skq = {}

    def load_chunk(k):
        st = skp.tile([128, CHI, 256], BF16, tag="skt", name=f"skt{k}")
        nc.sync.dma_start(out=st, in_=t["skT"][:, 4096 * k:4096 * (k + 1)])
        skq[k] = st

    for k in range(3):
        load_chunk(k)

    # ---- secondary consts on the gpsimd (Pool/SWDGE) ring ----
    wk = const.tile([128, 8, 128], BF16, tag="wk")
    xk = const.tile([128, 8, 256], BF16, tag="xk")
    wr = const.tile([128, 8, 128], BF16, tag="wr")
    xp = const.tile([128, 8, 256], BF16, tag="xp")
    wv = const.tile([128, 8, 128], BF16, tag="wv")
    xv = const.tile([128, 8, 256], BF16, tag="xv")
    wo = const.tile([128, 1024], BF16, tag="wo")
    mnot = const.tile([128, 4, 256], BF16, tag="mnot")
    for sb, name in ((wk, "wk"), (xk, "xk"), (wr, "wr"), (xp, "xp"),
                     (wv, "wv"), (xv, "xv"), (wo, "wo"), (mnot, "mnot")):
        nc.gpsimd.dma_start(out=sb, in_=t[name])

    # zero pads of the a3 bounce scratch (cols 256:512 of each (h,ib) plane)
    zt4 = const.tile([128, 4, 256], BF16, tag="zt4")
    nc.gpsimd.memset(zt4, 0.0)
    nc.gpsimd.dma_start(
        out=bass.AP(tensor=t["a3scr"].tensor, offset=t["a3scr"].offset + 256,
                    ap=[[512, 128], [65536, 4], [1, 256]]),
        in_=zt4)

    # PE warmup on the identity while DMAs land
    warm = psM.tile([128, 128], BF16, tag="pm", name="warm")
    for _ in range(14):
        nc.tensor.matmul(warm, ident, ident, start=True, stop=True,
                         is_transpose=True, skip_group_check=True)

    # B buffer: zero-spaced stationary, half-memsets on DVE
    B = const.tile([128, 8192], BF16, tag="B")
    Bv = B.rearrange("p (w c) -> p w c", c=128)
    nc.vector.memset(B[:, 0:4096], 0.0)

    # ---- projections (transposed): (128 d, 256 s) ----
    def proj_T(wsb, xsb, name):
        ps = psM.tile([128, 512], F32, tag="pm", name=name)
        for c in range(8):
            nc.tensor.matmul(ps[:, 0:256], wsb[:, c, :], xsb[:, c, :],
                             start=(c == 0), stop=(c == 7))
        return ps

    nqT = const.tile([128, 256], BF16, tag="nqT")
    nc.vector.tensor_scalar_add(out=nqT, in0=proj_T(wq, xq, "ps_nq")[:, 0:256],
                                scalar1=sq)
    nc.vector.memset(B[:, 4096:8192], 0.0)

    keyT = const.tile([128, 256], BF16, tag="keyT")
    nc.scalar.copy(out=keyT, in_=proj_T(wk, xk, "ps_key")[:, 0:256])

    def scatter_a(g):
        """live cols for windows w<32 of group g."""
        base = 64 * g
        nc.gpsimd.tensor_copy(out=Bv[0:64, 0:32, 0],
                              in_=nqT[0:64, base:base + 32])
        nc.gpsimd.tensor_copy(out=Bv[64:128, 0:32, 64],
                              in_=nqT[64:128, base:base + 32])

    def scatter_b(g):
        base = 64 * g
        nc.gpsimd.tensor_copy(out=Bv[0:64, 32:64, 0],
                              in_=nqT[0:64, base + 32:base + 64])
        nc.gpsimd.tensor_copy(out=Bv[64:128, 32:64, 64],
                              in_=nqT[64:128, base + 32:base + 64])

    S_ps = [None] * 4

    def open_group(g):
        base = 64 * g
        sp = psS.tile([128, 512], F32, tag="S", name=f"S{g}")
        S_ps[g] = sp
        for h in range(2):
            hsl = slice(64 * h, 64 * h + 64)
            nc.tensor.matmul(sp[hsl, 0:256], nqT[hsl, base:base + 64],
                             keyT[hsl, :], start=True, stop=False,
                             skip_group_check=True)

    def a2_run(g, w0, w1):
        base = 64 * g
        sp = S_ps[g]
        for w in range(w0, w1):
            i = base + w
            k, r = divmod(i, CHI)
            st = skq[k]
            nc.tensor.matmul(sp[:, 0:256], B[:, 127 * w:127 * w + 128],
                             st[:, r, :], start=False, stop=False,
                             skip_group_check=True)
            if r == CHI - 1:
                del skq[k]
                if k + 3 < NCHUNK:
                    load_chunk(k + 3)

    a3sb = None

    def a3_inject(g):
        """fold shifted a3 into the score PSUM; closes the accumulation."""
        sp = S_ps[g]
        qsl = slice(64 * (g % 2), 64 * (g % 2) + 64)
        for h in range(2):
            hsl = slice(64 * h, 64 * h + 64)
            nc.tensor.matmul(sp[hsl, 0:256], ident[qsl, qsl],
                             a3sb[qsl, h, g // 2, :], start=False,
                             stop=(h == 1), skip_group_check=True)

    # ---- a3: small matmuls + consolidated DRAM bounce ----
    def a3_emit(relT):
        nonlocal a3sb
        raw4 = work.tile([128, 4, 256], BF16, tag="raw4")
        for h in range(2):
            hsl = slice(64 * h, 64 * h + 64)
            for ib in range(2):
                ps = psM.tile([128, 512], F32, tag="pm", name=f"a3r{h}{ib}")
                nc.tensor.matmul(ps[:, 0:256], nqT[hsl, 128 * ib:128 * ib + 128],
                                 relT[hsl, :], start=True, stop=True)
                nc.scalar.copy(out=raw4[:, 2 * h + ib, :], in_=ps[:, 0:256])
        scr = t["a3scr"]
        nc.sync.dma_start(
            out=bass.AP(tensor=scr.tensor, offset=scr.offset,
                        ap=[[512, 128], [65536, 4], [1, 256]]),
            in_=raw4)
        sh = const.tile([128, 2, 2, 256], BF16, tag="a3sb")
        for h in range(2):
            nc.sync.dma_start(
                out=sh[:, h, :, :],
                in_=bass.AP(tensor=scr.tensor,
                            offset=scr.offset + 255 + 131072 * h,
                            ap=[[511, 128], [65408, 2], [1, 256]]))
        a3sb = sh

    # ---- per-group tail ----
    value = [None, None]
    sc_t = [None] * 4
    stc_t = [None] * 4
    aT_t = [None] * 4

    def tail_vec(g):
        sp = S_ps[g]
        w3 = work.tile([128, 256], F32, tag="w3")
        nc.vector.tensor_mul(out=w3, in0=sp[:, 0:256], in1=mnot[:, g, :])
        ex = work.tile([128, 256], F32, tag="ex")
        nc.scalar.activation(out=ex, in_=w3,
                             func=mybir.ActivationFunctionType.Exp, scale=1.0)
        den = work.tile([128, 1], F32, tag="den")
        nc.vector.reduce_sum(out=den, in_=ex, axis=mybir.AxisListType.X)
        rden = work.tile([128, 1], F32, tag="rden")
        nc.vector.reciprocal(out=rden, in_=den)
        sc = work.tile([128, 256], BF16, tag="sc", name=f"sc{g}")
        nc.vector.tensor_scalar_mul(out=sc, in0=ex, scalar1=rden)
        sc_t[g] = sc

    def tail_tr(g):
        sc = sc_t[g]
        stc = []
        for jb in range(2):
            tp = psT.tile([128, 128], BF16, tag="tp", name=f"tp{g}{jb}")
            nc.tensor.transpose(tp, sc[:, 128 * jb:128 * jb + 128], ident)
            st = work.tile([128, 128], BF16, tag="stc", name=f"stc{g}{jb}")
            nc.scalar.copy(out=st, in_=tp)
            stc.append(st)
        stc_t[g] = stc

    def tail_av(g):
        stc = stc_t[g]
        av = psM.tile([128, 512], F32, tag="pm", name=f"av{g}")
        for h in range(2):
            hsl = slice(64 * h, 64 * h + 64)
            for jb in range(2):
                nc.tensor.matmul(av[hsl, 0:64], value[jb][:, hsl],
                                 stc[jb][:, hsl], start=(jb == 0),
                                 stop=(jb == 1), skip_group_check=True)
        aT = work.tile([128, 64], BF16, tag="aT", name=f"aT{g}")
        nc.scalar.copy(out=aT, in_=av[:, 0:64])
        aT_t[g] = aT

    def tail_out(g):
        aT = aT_t[g]
        ob = work.tile([64, 1024], F32, tag="ob", name=f"ob{g}")
        for eh in range(2):
            op = psM.tile([128, 512], F32, tag="pm", name=f"op{g}{eh}")
            nc.tensor.matmul(op[0:64, :], aT, wo[:, 512 * eh:512 * (eh + 1)],
                             start=True, stop=True, skip_group_check=True)
            if eh == 0:
                nc.vector.tensor_copy(out=ob[:, 0:512], in_=op[0:64, :])
            else:
                nc.scalar.copy(out=ob[:, 512:1024], in_=op[0:64, :])
        nc.sync.dma_start(out=t["outp"][64 * g:64 * g + 64, :], in_=ob)

    # ---- schedule ----
    scatter_a(0)
    open_group(0)
    a2_run(0, 0, 16)

    relT = const.tile([128, 256], BF16, tag="relT")
    nc.scalar.copy(out=relT, in_=proj_T(wr, xp, "ps_rel")[:, 0:256])

    a2_run(0, 16, 32)
    scatter_b(0)
    a3_emit(relT)

    a2_run(0, 32, 48)

    for jb in range(2):
        ps = psM.tile([128, 512], F32, tag="pm", name=f"ps_val{jb}")
        for c in range(8):
            nc.tensor.matmul(ps[:, 0:128], xv[:, c, 128 * jb:128 * jb + 128],
                             wv[:, c, :], start=(c == 0), stop=(c == 7))
        vsb = const.tile([128, 128], BF16, tag=f"value{jb}")
        nc.scalar.copy(out=vsb, in_=ps[:, 0:128])
        value[jb] = vsb

    scatter_a(1)
    a2_run(0, 48, 64)
    a3_inject(0)
    tail_vec(0)

    open_group(1)
    a2_run(1, 0, 16)
    scatter_b(1)
    a2_run(1, 16, 32)
    tail_tr(0)
    scatter_a(2)
    a2_run(1, 32, 48)
    tail_av(0)
    a2_run(1, 48, 64)
    a3_inject(1)
    tail_vec(1)
    tail_out(0)

    for g in (2, 3):
        open_group(g)
        a2_run(g, 0, 16)
        scatter_b(g)
        a2_run(g, 16, 32)
        tail_tr(g - 1)
        if g < 3:
            scatter_a(g + 1)
        a2_run(g, 32, 48)
        tail_av(g - 1)
        a2_run(g, 48, 64)
        a3_inject(g)
        tail_vec(g)
        tail_out(g - 1)

    tail_tr(3)
    tail_av(3)
    tail_out(3)

    ctx.close()


def build():
    nc = bacc.Bacc("TRN2", target_bir_lowering=False, debug=False)
    t = {}

    def inp(name, shape, dt=BF16):
        t[name] = nc.dram_tensor(name, list(shape), dt, kind="ExternalInput").ap()

    inp("skT", (128, S * S))
    inp("sq", (128, 1), F32)
    for n in ("xq", "xk", "xp", "xv"):
        inp(n, (128, 8, 256))
    for n in ("wq", "wk", "wr", "wv"):
        inp(n, (128, 8, 128))
    inp("wo", (128, 1024))
    inp("mnot", (128, 4, 256))
    t["a3scr"] = nc.dram_tensor("a3scr", [4, 128, 512], BF16).ap()
    t["outp"] = nc.dram_tensor("outp", [S, E], F32, kind="ExternalOutput").ap()

    with tile.TileContext(nc) as tc:
        emit(tc, t)
    nc.compile()
    return nc


def make_in_maps(inputs: dict) -> list[dict]:
    q = np.asarray(inputs["q"], np.float32)
    k = np.asarray(inputs["k"], np.float32)
    v = np.asarray(inputs["v"], np.float32)
    p = np.asarray(inputs["p"], np.float32)
    mask = np.asarray(inputs["mask"])
    s_q = np.asarray(inputs["s_q"], np.float32)
    s_k = np.asarray(inputs["s_k"], np.float32)
    Wq = np.asarray(inputs["Wq"], np.float32)
    Wk = np.asarray(inputs["Wk"], np.float32)
    Wv = np.asarray(inputs["Wv"], np.float32)
    Wr = np.asarray(inputs["Wr"], np.float32)
    Wo = np.asarray(inputs["Wo"], np.float32)

    def actT(x):
        return np.ascontiguousarray(
            x.T.reshape(8, 128, 256).transpose(1, 0, 2)).astype(NPBF)

    xq, xk, xp, xv = actT(q), actT(k), actT(p), actT(v)

    maps = []
    for c in range(NCORES):
        rows = slice(c * DHB, (c + 1) * DHB)

        def wT(W):
            return np.ascontiguousarray(
                W[rows].T.reshape(8, 128, 128).transpose(1, 0, 2)).astype(NPBF)

        skT = np.ascontiguousarray(s_k[:, rows].T).astype(NPBF)
        mn = np.empty((128, 4, 256), np.float32)
        for g in range(4):
            for h in range(2):
                mn[64 * h:64 * h + 64, g] = (
                    1.0 - mask[2 * c + h, 64 * g:64 * g + 64].astype(np.float32)
                ) * SCALING
        maps.append({
            "skT": skT,
            "xq": xq, "xk": xk, "xp": xp, "xv": xv,
            "wq": wT(Wq), "wk": wT(Wk), "wr": wT(Wr), "wv": wT(Wv),
            "wo": np.ascontiguousarray(Wo[:, rows].T).astype(NPBF),
            "sq": np.ascontiguousarray(s_q[0, rows][:, None]),
            "mnot": mn.astype(NPBF),
        })
    return maps


_NC = None


def kernel(**inputs) -> np.ndarray:
    global _NC
    from concourse.bass_utils import run_bass_kernel_spmd

    if _NC is None:
        _NC = build()
    maps = make_in_maps(inputs)
    res = run_bass_kernel_spmd(_NC, maps, list(range(NCORES)))
    out = np.zeros((S, E), np.float32)
    for r in res.results:
        out += np.asarray(r["outp"], np.float32)
    return out


if __name__ == "__main__":
    nc = build()
    print("build ok")
